# revision 1
# baseline (speedup 1.0000x reference)
"""Trainium2 Bass kernel for the spiking CapsNet forward pass (nn_CapsNet).

Strategy (8 NeuronCores):
  Phase A (batch-parallel, 4 images/core):
    conv1 once (input is constant over the 5 timesteps), conv-layer membrane
    dynamics for all 5 steps upfront, then the expensive prim conv batched
    over all 5 timesteps in one weight-stationary PE pass.
  AllToAll: re-shard prim spikes from batch-split to route-split.
  Phase B (route-parallel, 144 routes/core, full batch):
    u_hat on PE per route (contraction over i=8), digit-caps membrane /
    trace / routing chain on DVE+GPSIMD with a small AllGather of the
    per-core s_j partials each step.
Host side: input re-layout (im2col, weight transposes) and the final
  classes = sqrt(sum_o (out_mem/T)^2) reduction.
"""

import numpy as np

N_CORES = 8
T = 5
B = 32
BL = B // N_CORES          # local batch (4)
R = 1152
RL = R // N_CORES          # local routes (144)
CO = 160                   # (o,c) pairs, ordered co = o*10 + c
DECAY = np.float32(0.2)
THRESH = np.float32(0.5)
DECAY_TR = np.float32(np.exp(np.float32(-1.0 / 1.5)))
ALPHA = np.float32(np.float32(0.0008) / np.float32(32.0))

_CACHE = {}


def _build_program(reps=1, stage="full", solo=False):
    import concourse.bass as bass
    import concourse.mybir as mybir
    import concourse.tile as tile
    from concourse import bacc

    Alu = mybir.AluOpType
    Act = mybir.ActivationFunctionType
    f32 = mybir.dt.float32

    nc = bacc.Bacc("TRN2", target_bir_lowering=False, debug=False,
                   num_devices=N_CORES)

    # ---- external I/O (per-core values supplied by host) ----
    im2_d = nc.dram_tensor("im2", [81, BL * 400], f32, kind="ExternalInput")
    convw_d = nc.dram_tensor("convw", [81, 256], f32, kind="ExternalInput")
    convb_d = nc.dram_tensor("convb", [128, 2], f32, kind="ExternalInput")
    primw_d = nc.dram_tensor("primw", [81, 128, 512], f32, kind="ExternalInput")
    primb_d = nc.dram_tensor("primb", [128, 2], f32, kind="ExternalInput")
    w2g_d = nc.dram_tensor("w2g", [9, 128, 16 * CO], f32,
                           kind="ExternalInput")
    thrm_d = nc.dram_tensor("thrm", [128, T], f32, kind="ExternalInput")
    thrt_d = nc.dram_tensor("thrt", [128, T], f32, kind="ExternalInput")
    sela_d = nc.dram_tensor("sela", [128, 160], f32, kind="ExternalInput")
    selt_d = nc.dram_tensor("selt", [32, 160], f32, kind="ExternalInput")
    outm_d = nc.dram_tensor("outm", [160, 32], f32, kind="ExternalOutput")
    dbg_d = nc.dram_tensor("dbg", [128, 512], f32, kind="ExternalOutput")

    # ---- internal DRAM ----
    # a2a: per-dest-rank blocks [dest 8][b_l 4][t 5][flat 1152]
    a2a_in = nc.dram_tensor("a2a_in", [N_CORES * BL * T * 1152], f32)
    a2a_out = nc.dram_tensor("a2a_out", [N_CORES * BL * T * 1152], f32)
    # per-step s_j exchange (payload: 4096 main + 4096 tail-partials)
    SJP = 8192
    sj_in = [nc.dram_tensor(f"sj_in{t}", [SJP], f32) for t in range(T)]
    sj_out = [nc.dram_tensor(f"sj_out{t}", [N_CORES * SJP], f32,
                             addr_space="Shared") for t in range(T)]

    def A(t, p0, pc, dims, foff=0):
        """Raw AP on tile/tensor t: partitions [p0, p0+pc), free dims list
        [(step, count), ...] in elements, plus extra free offset."""
        b = t if isinstance(t, bass.AP) else t[:]
        pitch = b.ap[0][0]
        return bass.AP(b.tensor, b.offset + p0 * pitch + foff,
                       [[pitch, pc]] + [list(d) for d in dims])

    def D(h, dims, off=0):
        """Raw AP on a DRAM tensor handle (flat element space)."""
        b = h[:]
        return bass.AP(b.tensor, off, [list(d) for d in dims])

    rg = [list(range(N_CORES))]

    with tile.TileContext(nc) as tc:
        from contextlib import ExitStack
        for _rep in range(reps):
            _run_once(nc, tc, bass, mybir, locals(), stage, solo)

    nc.compile()
    return nc


def _run_once(nc, tc, bass, mybir, env, stage="full", solo=False):
    import numpy as np
    from contextlib import ExitStack
    Alu = mybir.AluOpType
    f32 = mybir.dt.float32
    im2_d = env["im2_d"]; convw_d = env["convw_d"]; convb_d = env["convb_d"]
    primw_d = env["primw_d"]; primb_d = env["primb_d"]
    thrm_d = env["thrm_d"]; thrt_d = env["thrt_d"]; sela_d = env["sela_d"]
    selt_d = env["selt_d"]; outm_d = env["outm_d"]; dbg_d = env["dbg_d"]
    a2a_in = env["a2a_in"]; a2a_out = env["a2a_out"]
    w2g_d = env["w2g_d"]
    sj_in = env["sj_in"]; sj_out = env["sj_out"]; SJP = env["SJP"]
    A = env["A"]; D = env["D"]; rg = env["rg"]

    if True:
        with ExitStack() as stk:
            # ---------------- persistent pools ----------------
            persist = stk.enter_context(tc.tile_pool(name="persist", bufs=1))
            dram = stk.enter_context(tc.tile_pool(name="dram", bufs=1,
                                                  space="DRAM"))

            # =========== Phase A: conv stage (batch-sharded) ===========
            with ExitStack() as cstk:
                cpool = cstk.enter_context(tc.tile_pool(name="conv", bufs=1))
                wpool = cstk.enter_context(tc.tile_pool(name="wpos", bufs=4))
                cps = cstk.enter_context(tc.tile_pool(name="cpsum", bufs=4,
                                                      space="PSUM"))
                pps = cstk.enter_context(tc.tile_pool(name="ppsum", bufs=1,
                                                      space="PSUM"))

                IM = cpool.tile([81, BL * 400], f32, name="im", tag="im")
                CW = cpool.tile([81, 256], f32, name="cw", tag="cw")
                CB = cpool.tile([128, 2], f32, name="cb", tag="cb")
                PB = cpool.tile([128, 2], f32, name="pb", tag="pb")
                nc.sync.dma_start(out=IM[:], in_=im2_d[:])
                nc.sync.dma_start(out=CW[:], in_=convw_d[:])
                nc.sync.dma_start(out=CB[:], in_=convb_d[:])
                nc.sync.dma_start(out=PB[:], in_=primb_d[:])

                # SPIKES[kc] holds conv spikes for all (t, b): [128, 8000]
                SPIKES = [cpool.tile([128, T * BL * 400], f32, name=f"spk{kc}", tag=f"spk{kc}")
                          for kc in range(2)]
                CONVOUT = [cpool.tile([128, BL * 400], f32, name=f"co{kc}", tag=f"co{kc}")
                           for kc in range(2)]
                MPC = [cpool.tile([128, BL * 400], f32, name=f"mpc{kc}", tag=f"mpc{kc}")
                       for kc in range(2)]
                ASC = [cpool.tile([128, BL * 400], f32, name=f"asc{kc}", tag=f"asc{kc}")
                       for kc in range(2)]

                # --- conv1: out[co, (b,pix)] = sum_k convw[k,co] im2[k,(b,pix)]
                for mc in range(2):
                    for b in range(BL):
                        pc = cps.tile([128, 400], f32, name="cvp", tag="cvp")
                        nc.tensor.matmul(
                            out=pc[:, 0:400],
                            lhsT=CW[:, mc * 128:(mc + 1) * 128],
                            rhs=IM[:, b * 400:(b + 1) * 400],
                            start=True, stop=True)
                        # relu(x + bias): exact via DVE tensor_scalar chain
                        nc.vector.tensor_scalar(
                            out=CONVOUT[mc][:, b * 400:(b + 1) * 400],
                            in0=pc[:, 0:400],
                            scalar1=CB[:, mc:mc + 1], scalar2=0.0,
                            op0=Alu.add, op1=Alu.max)

                # --- conv membrane dynamics for all T steps ---
                for t in range(T):
                    for kc in range(2):
                        spk = A(SPIKES[kc], 0, 128, [(1, BL * 400)],
                                t * BL * 400)
                        if t == 0:
                            nc.vector.tensor_scalar(
                                out=spk, in0=CONVOUT[kc][:],
                                scalar1=1.0, scalar2=None, op0=Alu.is_gt)
                            nc.gpsimd.tensor_tensor(
                                out=MPC[kc][:], in0=CONVOUT[kc][:], in1=spk,
                                op=Alu.subtract)
                        else:
                            nc.vector.scalar_tensor_tensor(
                                out=ASC[kc][:], in0=MPC[kc][:], scalar=0.2,
                                in1=CONVOUT[kc][:],
                                op0=Alu.mult, op1=Alu.add)
                            nc.vector.tensor_scalar(
                                out=spk, in0=ASC[kc][:],
                                scalar1=1.0, scalar2=None, op0=Alu.is_gt)
                            if t < T - 1:
                                nc.gpsimd.tensor_tensor(
                                    out=MPC[kc][:], in0=ASC[kc][:], in1=spk,
                                    op=Alu.subtract)

                if stage == "convmem":
                    nc.sync.dma_start(out=D(outm_d, [(32, 128), (1, 32)]),
                                      in_=A(SPIKES[0], 0, 128, [(1, 32)]))
                    return
                # --- prim conv: batched over all (t, b):
                # out[co, (n=(t,b), oy, ox)] accumulated over (kc, ky, kx)
                PSP = [[pps.tile([128, 360], f32, name=f"pp{mc}{nch}", tag=f"pp{mc}{nch}")
                        for nch in range(2)] for mc in range(2)]
                for pos in range(81):
                    ky, kx = pos // 9, pos % 9
                    wt = wpool.tile([128, 512], f32, name="w", tag="w")
                    (nc.sync if pos % 2 == 0 else nc.scalar).dma_start(
                        out=wt[:],
                        in_=D(primw_d, [(512, 128), (1, 512)], pos * 128 * 512))
                    for kc in range(2):
                        for mc in range(2):
                            lhsT = wt[:, kc * 256 + mc * 128:
                                      kc * 256 + (mc + 1) * 128]
                            for nch in range(2):
                                rhs = A(SPIKES[kc], 0, 128,
                                        [(400, 10), (40, 6), (2, 6)],
                                        nch * 4000 + ky * 20 + kx)
                                nc.tensor.matmul(
                                    out=PSP[mc][nch][:, 0:360],
                                    lhsT=lhsT, rhs=rhs,
                                    start=(pos == 0 and kc == 0),
                                    stop=(pos == 80 and kc == 1))

                # --- prim evac (+bias), membranes, spikes ---
                PRIM = [cpool.tile([128, T * BL * 36], f32, name=f"pr{mc}", tag=f"pr{mc}")
                        for mc in range(2)]
                PSPK = [cpool.tile([128, T * BL * 36], f32, name=f"ps{mc}", tag=f"ps{mc}")
                        for mc in range(2)]
                MPP = [cpool.tile([128, BL * 36], f32, name=f"mpp{mc}", tag=f"mpp{mc}")
                       for mc in range(2)]
                APP = [cpool.tile([128, BL * 36], f32, name=f"app{mc}", tag=f"app{mc}")
                       for mc in range(2)]
                for mc in range(2):
                    for nch in range(2):
                        nc.vector.tensor_scalar(
                            out=PRIM[mc][:, nch * 360:(nch + 1) * 360],
                            in0=PSP[mc][nch][:, 0:360],
                            scalar1=PB[:, mc:mc + 1], scalar2=None,
                            op0=Alu.add)
                if stage == "prim":
                    nc.sync.dma_start(out=D(outm_d, [(32, 128), (1, 32)]),
                                      in_=A(PRIM[0], 0, 128, [(1, 32)]))
                    return
                for t in range(T):
                    for mc in range(2):
                        po = A(PRIM[mc], 0, 128, [(1, 144)], t * 144)
                        sp = A(PSPK[mc], 0, 128, [(1, 144)], t * 144)
                        if t == 0:
                            nc.vector.tensor_scalar(
                                out=sp, in0=po, scalar1=1.0, scalar2=None,
                                op0=Alu.is_gt)
                            nc.gpsimd.tensor_tensor(
                                out=MPP[mc][:], in0=po, in1=sp,
                                op=Alu.subtract)
                        else:
                            nc.vector.scalar_tensor_tensor(
                                out=APP[mc][:], in0=MPP[mc][:], scalar=0.2,
                                in1=po, op0=Alu.mult, op1=Alu.add)
                            nc.vector.tensor_scalar(
                                out=sp, in0=APP[mc][:], scalar1=1.0,
                                scalar2=None, op0=Alu.is_gt)
                            if t < T - 1:
                                nc.gpsimd.tensor_tensor(
                                    out=MPP[mc][:], in0=APP[mc][:], in1=sp,
                                    op=Alu.subtract)

                # --- prim spikes -> a2a_in ---
                # flat f = (mc*128+p)*36 + pix ; dest block j = f//1152,
                # rem = f%1152. Split partitions in groups of 32 so j is
                # constant per DMA:  p = ph*32 + pl ->
                # dst = j*23040 + b*5760 + t*1152 + pl*36 + pix, j = mc*4+ph
                env_spk0 = SPIKES[0]
                dmae = [nc.sync, nc.scalar, nc.gpsimd]
                di = 0
                for mc in range(2):
                    for ph in range(4):
                        j = mc * 4 + ph
                        for t in range(T):
                            src = A(PSPK[mc], ph * 32, 32,
                                    [(36, BL), (1, 36)], t * 144)
                            dst = D(a2a_in,
                                    [(36, 32), (5760, BL), (1, 36)],
                                    j * 23040 + t * 1152)
                            dmae[di % 3].dma_start(out=dst, in_=src)
                            di += 1

            if stage == "conv":
                nc.sync.dma_start(out=D(outm_d, [(32, 128), (1, 32)]),
                                  in_=A(env_spk0, 0, 128, [(1, 32)]))
                return
            # =========== AllToAll: batch-shard -> route-shard ===========
            if solo:
                nc.sync.dma_start(out=a2a_out[:], in_=a2a_in[:])
            else:
                nc.gpsimd.collective_compute(
                    "AllToAll", Alu.bypass, replica_groups=rg,
                    ins=[a2a_in[:]], outs=[a2a_out[:]])

            if stage == "a2a":
                nc.sync.dma_start(out=D(outm_d, [(32, 128), (1, 32)]),
                                  in_=D(a2a_out, [(32, 128), (1, 32)]))
                return
            # =========== Phase B prep: transposes + u_hat ===========
            uall = stk.enter_context(tc.tile_pool(name="uall", bufs=1))
            UH_m = uall.tile([128, RL * T * 32], f32, name="uhm", tag="uhm")
            UH_t = uall.tile([128, 36 * T * 32], f32, name="uht", tag="uht")
            with ExitStack() as ustk:
                upool = ustk.enter_context(tc.tile_pool(name="uh", bufs=1))
                w2pool = ustk.enter_context(tc.tile_pool(name="w2c", bufs=3))
                ups = ustk.enter_context(tc.tile_pool(name="upsum", bufs=4,
                                                      space="PSUM"))

                # Load a2a output as M[g] [32 b, (t 5, flat 128)] (contiguous
                # 512B runs), then PE-transpose 45 [32,128] blocks into
                # X16[g] [128 (rr,i), (t,b) 160].
                IDT = upool.tile([32, 32], f32, name="idt", tag="idt")
                from concourse.masks import make_identity
                make_identity(nc, IDT[:])
                X16 = [persist.tile([128, 160], f32, name=f"x16_{g}",
                                    tag=f"x16_{g}") for g in range(9)]
                for g in range(9):
                    Mg = upool.tile([32, T * 128], f32, name="mg", tag="mg",
                                    bufs=2)
                    nc.sync.dma_start(
                        out=Mg[:],
                        in_=D(a2a_out, [(5760, 32), (1152, T), (1, 128)],
                              g * 128))
                    for t in range(T):
                        pst = ups.tile([128, 32], f32, name="pst", tag="pst", bufs=2)
                        nc.tensor.transpose(
                            out=pst[:], in_=Mg[:, t * 128:(t + 1) * 128],
                            identity=IDT[:])
                        nc.vector.tensor_copy(
                            out=X16[g][:, t * 32:(t + 1) * 32], in_=pst[:])

                if stage == "trans":
                    nc.sync.dma_start(
                        out=D(outm_d, [(32, 128), (1, 32)]),
                        in_=A(X16[0], 0, 128, [(1, 32)]))
                    return
                # u_hat per local route r, kept SBUF-resident for all t:
                #   UH_m [128 co, (r 144, t 5, b 32)]
                #   UH_t [128 (rq,cot), (rl 36, t 5, b 32)]
                # K=32 matmuls with zero-padded weights (partition-aligned);
                # tail matmul writes PSUM at col-group rq*32 so the evac
                # copy is partition-aligned too.
                for g in range(9):
                    w2c = w2pool.tile([128, 16 * CO], f32, name="w2c",
                                      tag="w2c", bufs=2)
                    (nc.sync if g % 2 == 0 else nc.scalar).dma_start(
                        out=w2c[:],
                        in_=D(w2g_d, [(2560, 128), (1, 2560)],
                              g * 128 * 2560))
                    for r2 in range(8):
                        psA = ups.tile([128, 320], f32, name="upa", tag="upa",
                                       bufs=3)
                        psB = ups.tile([128, 320], f32, name="upb", tag="upb",
                                       bufs=3)
                        for j in range(2):
                            rr = r2 * 2 + j
                            r = g * 16 + rr
                            rq = r // 36
                            q = (rr // 4) * 32
                            rhs = A(X16[g], q, 32, [(1, 160)])
                            nc.tensor.matmul(
                                out=psA[:, j * 160:(j + 1) * 160],
                                lhsT=A(w2c, q, 32, [(1, 128)], rr * CO),
                                rhs=rhs, start=True, stop=True,
                                tile_position=(q, 0))
                            nc.tensor.matmul(
                                out=A(psB, rq * 32, 32, [(1, 160)], j * 160),
                                lhsT=A(w2c, q, 32, [(1, 32)], rr * CO + 128),
                                rhs=rhs, start=True, stop=True,
                                tile_position=(q, rq * 32))
                        r0 = g * 16 + r2 * 2
                        rq0, rl0 = r0 // 36, r0 % 36
                        if r2 % 2 == 0:
                            nc.vector.tensor_copy(
                                out=A(UH_m, 0, 128, [(1, 320)], r0 * 160),
                                in_=psA[:, 0:320])
                            nc.scalar.copy(
                                out=A(UH_t, rq0 * 32, 32, [(1, 320)],
                                      rl0 * 160),
                                in_=A(psB, rq0 * 32, 32, [(1, 320)]))
                        else:
                            nc.scalar.copy(
                                out=A(UH_m, 0, 128, [(1, 320)], r0 * 160),
                                in_=psA[:, 0:320])
                            nc.vector.tensor_copy(
                                out=A(UH_t, rq0 * 32, 32, [(1, 320)],
                                      rl0 * 160),
                                in_=A(psB, rq0 * 32, 32, [(1, 320)]))

            if stage == "uhat":
                nc.sync.dma_start(out=D(outm_d, [(32, 128), (1, 32)]),
                                  in_=A(X16[0], 0, 128, [(1, 32)]))
                return
            # =========== Phase B: digit-caps loop (route-sharded) ========
            dpool = stk.enter_context(tc.tile_pool(name="dig", bufs=1))
            dups = stk.enter_context(tc.tile_pool(name="dups", bufs=2))
            dps = stk.enter_context(tc.tile_pool(name="dpsum", bufs=2,
                                                 space="PSUM"))

            NM = RL * 32            # 4608
            NTT = 36 * 32           # 1152
            THRM = dpool.tile([128, T], f32, name="thrm", tag="thrm")
            THRT = dpool.tile([128, T], f32, name="thrt", tag="thrt")
            SELA = dpool.tile([128, 160], f32, name="sela", tag="sela")
            SELT = dpool.tile([32, 160], f32, name="selt", tag="selt")
            nc.sync.dma_start(out=THRM[:], in_=thrm_d[:])
            nc.sync.dma_start(out=THRT[:], in_=thrt_d[:])
            nc.sync.dma_start(out=SELA[:], in_=sela_d[:])
            nc.sync.dma_start(out=SELT[:], in_=selt_d[:])

            MD_m = dpool.tile([128, NM], f32, name="mdm", tag="mdm")
            MD_t = dpool.tile([128, NTT], f32, name="mdt", tag="mdt")
            DS_m = dpool.tile([128, NM], f32, name="dsm", tag="dsm")
            DS_t = dpool.tile([128, NTT], f32, name="dst", tag="dst")
            TR_m = dpool.tile([128, NM], f32, name="trm", tag="trm")
            TR_t = dpool.tile([128, NTT], f32, name="trt", tag="trt")
            BIJ_m = dpool.tile([128, RL], f32, name="bijm", tag="bijm")
            BIJ_t = dpool.tile([128, 36], f32, name="bijt", tag="bijt")
            ZB_m = dpool.tile([128, RL], f32, name="zbm", tag="zbm")
            ZB_t = dpool.tile([128, 36], f32, name="zbt", tag="zbt")
            SJQ = dpool.tile([128, 32], f32, name="sjq", tag="sjq")
            SJF_m = dpool.tile([128, 32], f32, name="sjfm", tag="sjfm")
            SJF_t = dpool.tile([32, 32], f32, name="sjft", tag="sjft")
            A2_m = dpool.tile([128, 32], f32, name="a2m", tag="a2m")
            A2_t = dpool.tile([32, 32], f32, name="a2t", tag="a2t")
            M2_m = dpool.tile([128, 32], f32, name="m2m", tag="m2m")
            M2_t = dpool.tile([32, 32], f32, name="m2t", tag="m2t")
            D2_m = dpool.tile([128, 32], f32, name="d2m", tag="d2m")
            D2_t = dpool.tile([32, 32], f32, name="d2t", tag="d2t")
            D2F = dpool.tile([128, 32], f32, name="d2f", tag="d2f")
            DBG = dpool.tile([128, 512], f32, name="dbg", tag="dbg")
            OUT_m = dpool.tile([128, 32], f32, name="outm", tag="outm")
            OUT_t = dpool.tile([32, 32], f32, name="outt", tag="outt")
            DSU_m = dpool.tile([128, 1], f32, name="dsum", tag="dsum")
            DSU_t = dpool.tile([32, 1], f32, name="dsut", tag="dsut")
            SCR_m = dpool.tile([128, 32], f32, name="scrm", tag="scrm")
            SCR_t = dpool.tile([32, 32], f32, name="scrt", tag="scrt")
            DPDF = dpool.tile([128, 36], f32, name="dpdf", tag="dpdf")
            SCOLF = dpool.tile([128, 1], f32, name="scolf", tag="scolf")

            bij0 = float(np.float32(1.0) / np.float32(R))
            nc.vector.memset(BIJ_m[:], bij0)
            nc.vector.memset(BIJ_t[:], bij0)

            for t in range(T):
                # ---- u_hat slices for this step (strided SBUF views) ----
                Um = A(UH_m, 0, 128, [(5 * 32, RL), (1, 32)], t * 32)
                Ut = A(UH_t, 0, 128, [(5 * 32, 36), (1, 32)], t * 32)
                DSm = A(DS_m, 0, 128, [(32, RL), (1, 32)])
                DSt = A(DS_t, 0, 128, [(32, 36), (1, 32)])
                MDm = A(MD_m, 0, 128, [(32, RL), (1, 32)])
                MDt = A(MD_t, 0, 128, [(32, 36), (1, 32)])

                # ---- membrane a-pass (in place on U), spikes, reset ----
                if t > 0:
                    nc.vector.scalar_tensor_tensor(
                        out=Um, in0=MDm, scalar=0.2, in1=Um,
                        op0=Alu.mult, op1=Alu.add)
                    nc.vector.scalar_tensor_tensor(
                        out=Ut, in0=MDt, scalar=0.2, in1=Ut,
                        op0=Alu.mult, op1=Alu.add)
                nc.vector.tensor_scalar(
                    out=DSm, in0=Um, scalar1=THRM[:, t:t + 1],
                    scalar2=None, op0=Alu.is_gt)
                nc.vector.tensor_scalar(
                    out=DSt, in0=Ut, scalar1=THRT[:, t:t + 1],
                    scalar2=None, op0=Alu.is_gt)
                if t < T - 1:
                    nc.gpsimd.tensor_tensor(
                        out=MDm, in0=Um, in1=DSm, op=Alu.subtract)
                    nc.gpsimd.tensor_tensor(
                        out=MDt, in0=Ut, in1=DSt, op=Alu.subtract)

                # ---- trace update (needed for steps 0..T-2) ----
                if t == 0:
                    nc.scalar.copy(out=TR_m[:], in_=DS_m[:])
                    nc.scalar.copy(out=TR_t[:], in_=DS_t[:])
                elif t < T - 1:
                    nc.vector.scalar_tensor_tensor(
                        out=TR_m[:], in0=TR_m[:], scalar=float(DECAY_TR),
                        in1=DS_m[:], op0=Alu.mult, op1=Alu.max)
                    nc.vector.scalar_tensor_tensor(
                        out=TR_t[:], in0=TR_t[:], scalar=float(DECAY_TR),
                        in1=DS_t[:], op0=Alu.mult, op1=Alu.max)

                # ---- y = ds * bij ; s_j partials (reduce over r) ----
                nc.vector.tensor_tensor(
                    out=DSm, in0=DSm,
                    in1=A(BIJ_m, 0, 128, [(1, RL), (0, 32)]),
                    op=Alu.mult)
                nc.vector.tensor_tensor(
                    out=DSt, in0=DSt,
                    in1=A(BIJ_t, 0, 128, [(1, 36), (0, 32)]),
                    op=Alu.mult)
                nc.vector.tensor_reduce(
                    out=SJF_m[:], in_=A(DS_m, 0, 128, [(1, 32), (32, RL)]),
                    axis=mybir.AxisListType.X, op=Alu.add)
                nc.vector.tensor_reduce(
                    out=SJQ[:], in_=A(DS_t, 0, 128, [(1, 32), (32, 36)]),
                    axis=mybir.AxisListType.X, op=Alu.add)

                # ---- exchange s_j partials (AllGather + local sum) ----
                nc.sync.dma_start(out=D(sj_in[t], [(32, 128), (1, 32)]),
                                  in_=SJF_m[:])
                nc.sync.dma_start(out=D(sj_in[t], [(32, 128), (1, 32)], 4096),
                                  in_=SJQ[:])
                if solo:
                    nc.sync.dma_start(
                        out=D(sj_out[t], [(1, SJP)]), in_=sj_in[t][:])
                else:
                    nc.gpsimd.collective_compute(
                        "AllGather", Alu.bypass, replica_groups=rg,
                        ins=[sj_in[t][:]], outs=[sj_out[t][:]])
                SJG_m = dups.tile([128, 8 * 32], f32, name="sjgm", tag="sjgm")
                SJG_t = dups.tile([32, 4 * 8 * 32], f32, name="sjgt", tag="sjgt")
                nc.sync.dma_start(
                    out=A(SJG_m, 0, 128, [(32, 8), (1, 32)]),
                    in_=D(sj_out[t], [(32, 128), (SJP, 8), (1, 32)]))
                for rq in range(4):
                    nc.sync.dma_start(
                        out=A(SJG_t, 0, 32, [(128, 8), (1, 32)], rq * 32),
                        in_=D(sj_out[t], [(32, 32), (SJP, 8), (1, 32)],
                              4096 + rq * 1024))
                nc.vector.tensor_reduce(
                    out=SJF_m[:], in_=A(SJG_m, 0, 128, [(1, 32), (32, 8)]),
                    axis=mybir.AxisListType.X, op=Alu.add)
                nc.vector.tensor_reduce(
                    out=SJF_t[:],
                    in_=A(SJG_t, 0, 32, [(1, 32), (32, 8 * 4)]),
                    axis=mybir.AxisListType.X, op=Alu.add)

                # ---- dig2 membranes, out accumulation ----
                if t == 0:
                    a2m, a2t = SJF_m, SJF_t
                    nc.vector.tensor_copy(out=OUT_m[:], in_=SJF_m[:])
                    nc.vector.tensor_copy(out=OUT_t[:], in_=SJF_t[:])
                else:
                    nc.vector.scalar_tensor_tensor(
                        out=A2_m[:], in0=M2_m[:], scalar=0.2, in1=SJF_m[:],
                        op0=Alu.mult, op1=Alu.add)
                    nc.vector.scalar_tensor_tensor(
                        out=A2_t[:], in0=M2_t[:], scalar=0.2, in1=SJF_t[:],
                        op0=Alu.mult, op1=Alu.add)
                    a2m, a2t = A2_m, A2_t
                    nc.vector.tensor_tensor(out=OUT_m[:], in0=OUT_m[:],
                                            in1=SJF_m[:], op=Alu.add)
                    nc.vector.tensor_tensor(out=OUT_t[:], in0=OUT_t[:],
                                            in1=SJF_t[:], op=Alu.add)
                if t < T - 1:
                    nc.vector.tensor_scalar(
                        out=D2_m[:], in0=a2m[:], scalar1=0.5, scalar2=None,
                        op0=Alu.is_gt)
                    nc.vector.tensor_scalar(
                        out=D2_t[:], in0=a2t[:], scalar1=0.5, scalar2=None,
                        op0=Alu.is_gt)
                    nc.vector.scalar_tensor_tensor(
                        out=M2_m[:], in0=D2_m[:], scalar=-0.5, in1=a2m[:],
                        op0=Alu.mult, op1=Alu.add)
                    nc.vector.scalar_tensor_tensor(
                        out=M2_t[:], in0=D2_t[:], scalar=-0.5, in1=a2t[:],
                        op0=Alu.mult, op1=Alu.add)

                    # d2s tail-folded copy (partition realign via DMA)
                    for rq in range(4):
                        nc.sync.dma_start(
                            out=A(D2F, rq * 32, 32, [(1, 32)]),
                            in_=D2_t[:])

                    # ---- z = trace * d2s (onto DS); zb = sum_b z ----
                    nc.vector.tensor_tensor(
                        out=DSm,
                        in0=A(TR_m, 0, 128, [(32, RL), (1, 32)]),
                        in1=A(D2_m, 0, 128, [(0, RL), (1, 32)]),
                        op=Alu.mult)
                    nc.vector.tensor_tensor(
                        out=DSt,
                        in0=A(TR_t, 0, 128, [(32, 36), (1, 32)]),
                        in1=A(D2F, 0, 128, [(0, 36), (1, 32)]),
                        op=Alu.mult)
                    nc.vector.tensor_reduce(
                        out=ZB_m[:], in_=A(DS_m, 0, 128, [(32, RL), (1, 32)]),
                        axis=mybir.AxisListType.X, op=Alu.add)
                    nc.vector.tensor_reduce(
                        out=ZB_t[:], in_=A(DS_t, 0, 128, [(32, 36), (1, 32)]),
                        axis=mybir.AxisListType.X, op=Alu.add)

                    # scaled d2s sums for the -0.1*alpha*S[c] term
                    nc.vector.tensor_scalar(
                        out=SCR_m[:], in0=D2_m[:],
                        scalar1=float(np.float32(0.1) * ALPHA), scalar2=None,
                        op0=Alu.mult, op1=Alu.add, accum_out=DSU_m[:])
                    nc.vector.tensor_scalar(
                        out=SCR_t[:], in0=D2_t[:],
                        scalar1=float(np.float32(0.1) * ALPHA), scalar2=None,
                        op0=Alu.mult, op1=Alu.add, accum_out=DSU_t[:])

                    if t == 0:
                        nc.vector.tensor_copy(out=DBG[:, 0:144],
                                              in_=ZB_m[:])
                        nc.vector.tensor_copy(out=DBG[:, 144:180],
                                              in_=ZB_t[:])
                    # ---- delta matmuls: PD[co', r] = sum_co sel * zb ----
                    PD_m = dps.tile([128, 145], f32, name="pdm", tag="pdm")
                    PD_t = dps.tile([32, 145], f32, name="pdt", tag="pdt")
                    nc.tensor.matmul(out=PD_m[:, 0:144],
                                     lhsT=SELA[:, 0:128], rhs=ZB_m[:],
                                     start=True, stop=False)
                    if t == 0:
                        nc.vector.tensor_copy(out=DBG[:, 145:289], in_=PD_m[:, 0:144])
                    nc.tensor.matmul(out=PD_m[:, 144:145],
                                     lhsT=SELA[:, 0:128], rhs=DSU_m[:],
                                     start=True, stop=False)
                    nc.tensor.matmul(out=PD_t[:, 0:144],
                                     lhsT=SELA[:, 128:160], rhs=ZB_m[:],
                                     start=True, stop=False)
                    nc.tensor.matmul(out=PD_t[:, 144:145],
                                     lhsT=SELA[:, 128:160], rhs=DSU_m[:],
                                     start=True, stop=False)
                    if t == 0:
                        nc.vector.tensor_copy(out=DBG[:, 180:212],
                                              in_=D2_m[:])
                        nc.vector.tensor_copy(out=DBG[0:32, 212:244],
                                              in_=D2_t[:])
                    # partition-compress ZB_t [128 (rq,cot), 36] ->
                    # ZBT4 [32 cot, (rq, 36)] so the matmul K sits at base 0
                    ZBT4 = dups.tile([32, 144], f32, name="zbt4", tag="zbt4")
                    for rq in range(4):
                        nc.sync.dma_start(
                            out=A(ZBT4, 0, 32, [(1, 36)], rq * 36),
                            in_=A(ZB_t, rq * 32, 32, [(1, 36)]))
                    nc.tensor.matmul(out=PD_m[:, 0:144],
                                     lhsT=SELT[:, 0:128], rhs=ZBT4[:],
                                     start=False, stop=False)
                    nc.tensor.matmul(out=PD_t[:, 0:144],
                                     lhsT=SELT[:, 128:160], rhs=ZBT4[:],
                                     start=False, stop=False)
                    nc.tensor.matmul(out=PD_m[:, 144:145],
                                     lhsT=SELT[:, 0:128], rhs=DSU_t[:],
                                     start=False, stop=True)
                    nc.tensor.matmul(out=PD_t[:, 144:145],
                                     lhsT=SELT[:, 128:160], rhs=DSU_t[:],
                                     start=False, stop=True)

                    if t == 0:
                        nc.vector.tensor_copy(out=DBG[:, 0:144],
                                              in_=PD_m[:, 0:144])
                        nc.vector.tensor_copy(out=DBG[:, 144:145],
                                              in_=PD_m[:, 144:145])
                    # ---- bij updates ----
                    nc.vector.scalar_tensor_tensor(
                        out=BIJ_m[:], in0=PD_m[:, 0:144], scalar=float(ALPHA),
                        in1=BIJ_m[:], op0=Alu.mult, op1=Alu.add)
                    nc.vector.tensor_scalar(
                        out=BIJ_m[:], in0=BIJ_m[:],
                        scalar1=PD_m[:, 144:145], scalar2=None,
                        op0=Alu.subtract)
                    # tail: fold [32 co', 144 r] -> [128 (rq,cot), 36]
                    PDTS = dups.tile([32, 145], f32, name="pdts", tag="pdts")
                    nc.vector.tensor_copy(out=PDTS[:], in_=PD_t[:])
                    for rq in range(4):
                        nc.sync.dma_start(
                            out=A(DPDF, rq * 32, 32, [(1, 36)]),
                            in_=A(PDTS, 0, 32, [(1, 36)], rq * 36))
                        nc.sync.dma_start(
                            out=A(SCOLF, rq * 32, 32, [(1, 1)]),
                            in_=PDTS[:, 144:145])
                    nc.vector.scalar_tensor_tensor(
                        out=BIJ_t[:], in0=DPDF[:], scalar=float(ALPHA),
                        in1=BIJ_t[:], op0=Alu.mult, op1=Alu.add)
                    nc.vector.tensor_scalar(
                        out=BIJ_t[:], in0=BIJ_t[:], scalar1=SCOLF[:],
                        scalar2=None, op0=Alu.subtract)

            nc.sync.dma_start(out=dbg_d[:], in_=DBG[:])
            # ---- write outputs ----
            nc.sync.dma_start(out=D(outm_d, [(32, 128), (1, 32)]),
                              in_=OUT_m[:])
            nc.sync.dma_start(out=D(outm_d, [(32, 32), (1, 32)], 128 * 32),
                              in_=OUT_t[:])


def _host_prepare(data, conv_w, conv_b, prim_w, prim_b, W, bias):
    """Build per-core input maps."""
    from numpy.lib.stride_tricks import sliding_window_view
    f32 = np.float32
    data = np.asarray(data, f32)
    conv_w = np.asarray(conv_w, f32)
    conv_b = np.asarray(conv_b, f32)
    prim_w = np.asarray(prim_w, f32)
    prim_b = np.asarray(prim_b, f32)
    W = np.asarray(W, f32)
    bias = np.asarray(bias, f32)

    # im2col: win[b, ky, kx, oy, ox]
    win = sliding_window_view(data[:, 0, :, :], (20, 20), axis=(1, 2))
    im2_all = np.ascontiguousarray(win).reshape(B, 81, 400)

    # everything feeding the spiking membranes runs in a 2x-scaled domain
    # (exact in fp32) so the reset is the plain subtract M = A - ds.
    convw = np.ascontiguousarray(conv_w.reshape(256, 81).T) * f32(2.0)
    convb2 = np.ascontiguousarray(conv_b.reshape(2, 128).T) * f32(2.0)

    pw = prim_w.reshape(2, 128, 2, 128, 9, 9)
    primw = np.ascontiguousarray(
        pw.transpose(4, 5, 3, 2, 0, 1).reshape(81, 128, 512)) * f32(2.0)
    primb2 = np.ascontiguousarray(prim_b.reshape(2, 128).T) * f32(2.0)

    # W2[i, r, co] with co = o*10 + c, zero-padded to K=32 route-quads:
    # w2g[g, rr*8+i, rr*160+co] = 2*W2[i, g*16+rr, co]
    Wt = np.ascontiguousarray(
        W.transpose(3, 0, 2, 1)).reshape(8, R, CO) * f32(2.0)

    # per-step thresholds: thr_t[co] = 0.5 - b_t[o], b_t = 0.2*b_{t-1}+bias_o
    bias_o = bias[:, 0]
    thr = np.zeros((CO, T), f32)
    bt = bias_o.copy()
    for t in range(T):
        for co in range(CO):
            thr[co, t] = np.float32(1.0) - f32(2.0) * bt[co // 10]
        bt = (f32(0.2) * bt + bias_o).astype(f32)
    thrm = np.ascontiguousarray(thr[:128])
    # tail thresholds, folded to 128 partitions (cot = p % 32)
    thrt = np.zeros((128, T), f32)
    for p in range(128):
        thrt[p] = thr[128 + p % 32]

    cos = np.arange(CO)
    sela = (np.equal.outer(cos[:128] % 10, cos % 10)).astype(f32)
    selt = (np.equal.outer(cos[128:] % 10, cos % 10)).astype(f32)
    sela = np.ascontiguousarray(sela)
    selt = np.ascontiguousarray(selt)

    in_maps = []
    for k in range(N_CORES):
        im2 = np.ascontiguousarray(
            im2_all[BL * k:BL * (k + 1)].transpose(1, 0, 2).reshape(81, 1600))
        w2core = Wt[:, RL * k:RL * (k + 1), :]          # [8, 144, 160]
        w2g = np.zeros((9, 128, 16 * CO), f32)
        for rr in range(16):
            # [8, 9, 160] block for this rr across all 9 groups
            blk = w2core[:, rr::16, :]
            w2g[:, rr * 8:(rr + 1) * 8, rr * CO:(rr + 1) * CO] = \
                blk.transpose(1, 0, 2)
        in_maps.append({
            "im2": im2, "convw": convw, "convb": convb2,
            "primw": primw, "primb": primb2, "w2g": w2g,
            "thrm": thrm, "thrt": thrt, "sela": sela, "selt": selt,
        })
    return in_maps


def _postprocess(outm):
    """outm [160, 32] (co = o*10+c) -> classes [32, 10]."""
    out3 = outm.reshape(16, 10, 32).astype(np.float32) / np.float32(T)
    sq = (out3 * out3).sum(axis=0)
    return np.sqrt(sq).T.astype(np.float32)


def kernel(data, conv_w, conv_b, prim_w, prim_b, W, bias, time_window):
    from concourse.bass_utils import run_bass_kernel_spmd
    assert int(time_window) == T
    if "nc" not in _CACHE:
        _CACHE["nc"] = _build_program()
    nc = _CACHE["nc"]
    in_maps = _host_prepare(data, conv_w, conv_b, prim_w, prim_b, W, bias)
    res = run_bass_kernel_spmd(nc, in_maps, core_ids=list(range(N_CORES)))
    return _postprocess(res.results[0]["outm"])



# revision 13
# speedup vs baseline: 8.6587x; 8.6587x over previous
"""Trainium2 Bass kernel for the spiking CapsNet forward pass (nn_CapsNet).

Strategy (8 NeuronCores):
  Phase A (batch-parallel, 4 images/core):
    conv1 once (input is constant over the 5 timesteps), conv-layer membrane
    dynamics for all 5 steps upfront, then the expensive prim conv batched
    over all 5 timesteps in one weight-stationary PE pass.
  AllToAll: re-shard prim spikes from batch-split to route-split.
  Phase B (route-parallel, 144 routes/core, full batch):
    u_hat on PE per route (contraction over i=8), digit-caps membrane /
    trace / routing chain on DVE+GPSIMD with a small AllGather of the
    per-core s_j partials each step.
Host side: input re-layout (im2col, weight transposes) and the final
  classes = sqrt(sum_o (out_mem/T)^2) reduction.
"""

import numpy as np

N_CORES = 8
T = 5
B = 32
BL = B // N_CORES          # local batch (4)
R = 1152
RL = R // N_CORES          # local routes (144)
CO = 160                   # (o,c) pairs, ordered co = o*10 + c
DECAY = np.float32(0.2)
THRESH = np.float32(0.5)
DECAY_TR = np.float32(np.exp(np.float32(-1.0 / 1.5)))
ALPHA = np.float32(np.float32(0.0008) / np.float32(32.0))

_CACHE = {}


def _build_program(reps=1, stage="full", solo=False):
    import concourse.bass as bass
    import concourse.mybir as mybir
    import concourse.tile as tile
    from concourse import bacc

    Alu = mybir.AluOpType
    Act = mybir.ActivationFunctionType
    f32 = mybir.dt.float32
    bf16 = mybir.dt.bfloat16

    nc = bacc.Bacc("TRN2", target_bir_lowering=False, debug=False,
                   num_devices=N_CORES)

    # ---- external I/O (per-core values supplied by host) ----
    im2_d = nc.dram_tensor("im2", [81, BL * 400], f32, kind="ExternalInput")
    convw_d = nc.dram_tensor("convw", [81, 256], f32, kind="ExternalInput")
    convb_d = nc.dram_tensor("convb", [128, 2], f32, kind="ExternalInput")
    primw_d = nc.dram_tensor("primw", [81, 128, 512], bf16,
                             kind="ExternalInput")
    primb_d = nc.dram_tensor("primb", [128, 2], f32, kind="ExternalInput")
    w2g_d = nc.dram_tensor("w2g", [9, 128, 16 * CO], bf16,
                           kind="ExternalInput")
    thrm_d = nc.dram_tensor("thrm", [128, T], f32, kind="ExternalInput")
    thrt_d = nc.dram_tensor("thrt", [128, T], f32, kind="ExternalInput")
    sela_d = nc.dram_tensor("sela", [128, 160], f32, kind="ExternalInput")
    selt_d = nc.dram_tensor("selt", [32, 160], f32, kind="ExternalInput")
    outm_d = nc.dram_tensor("outm", [160, 32], f32, kind="ExternalOutput")
    dbg_d = nc.dram_tensor("dbg", [128, 512], f32, kind="ExternalOutput")

    # ---- internal DRAM ----
    # a2a: per-dest-rank blocks [dest 8][b_l 4][t 5][flat 1152]
    a2a_in = nc.dram_tensor("a2a_in", [N_CORES * BL * T * 1152], bf16)
    a2a_out = nc.dram_tensor("a2a_out", [N_CORES * BL * T * 1152], bf16)
    # per-step s_j exchange (payload: 4096 main + 4096 tail-partials)
    SJP = 8192
    sj_in = [nc.dram_tensor(f"sj_in{t}", [SJP], f32) for t in range(T)]
    sj_out = [nc.dram_tensor(f"sj_out{t}", [N_CORES * SJP], f32,
                             addr_space="Shared") for t in range(T)]

    def A(t, p0, pc, dims, foff=0):
        """Raw AP on tile/tensor t: partitions [p0, p0+pc), free dims list
        [(step, count), ...] in elements, plus extra free offset."""
        b = t if isinstance(t, bass.AP) else t[:]
        pitch = b.ap[0][0]
        return bass.AP(b.tensor, b.offset + p0 * pitch + foff,
                       [[pitch, pc]] + [list(d) for d in dims])

    def D(h, dims, off=0):
        """Raw AP on a DRAM tensor handle (flat element space)."""
        b = h[:]
        return bass.AP(b.tensor, off, [list(d) for d in dims])

    rg = [list(range(N_CORES))]

    with tile.TileContext(nc) as tc:
        from contextlib import ExitStack
        for _rep in range(reps):
            _run_once(nc, tc, bass, mybir, locals(), stage, solo)

    nc.compile()
    return nc


def _run_once(nc, tc, bass, mybir, env, stage="full", solo=False):
    import numpy as np
    from contextlib import ExitStack
    Alu = mybir.AluOpType
    f32 = mybir.dt.float32
    bf16 = mybir.dt.bfloat16
    im2_d = env["im2_d"]; convw_d = env["convw_d"]; convb_d = env["convb_d"]
    primw_d = env["primw_d"]; primb_d = env["primb_d"]
    thrm_d = env["thrm_d"]; thrt_d = env["thrt_d"]; sela_d = env["sela_d"]
    selt_d = env["selt_d"]; outm_d = env["outm_d"]; dbg_d = env["dbg_d"]
    a2a_in = env["a2a_in"]; a2a_out = env["a2a_out"]
    w2g_d = env["w2g_d"]
    sj_in = env["sj_in"]; sj_out = env["sj_out"]; SJP = env["SJP"]
    A = env["A"]; D = env["D"]; rg = env["rg"]

    if True:
        with ExitStack() as stk:
            # ---------------- persistent pools ----------------
            persist = stk.enter_context(tc.tile_pool(name="persist", bufs=1))
            dram = stk.enter_context(tc.tile_pool(name="dram", bufs=1,
                                                  space="DRAM"))

            # =========== Phase A: conv stage (batch-sharded) ===========
            with ExitStack() as cstk:
                cpool = cstk.enter_context(tc.tile_pool(name="conv", bufs=1))
                wpool = cstk.enter_context(tc.tile_pool(name="wpos", bufs=4))
                cps = cstk.enter_context(tc.tile_pool(name="cpsum", bufs=4,
                                                      space="PSUM"))
                pps = cstk.enter_context(tc.tile_pool(name="ppsum", bufs=1,
                                                      space="PSUM"))

                IM = cpool.tile([81, BL * 400], f32, name="im", tag="im")
                CW = cpool.tile([81, 256], f32, name="cw", tag="cw")
                CB = cpool.tile([128, 2], f32, name="cb", tag="cb")
                PB = cpool.tile([128, 2], f32, name="pb", tag="pb")
                nc.sync.dma_start(out=IM[:], in_=im2_d[:])
                nc.sync.dma_start(out=CW[:], in_=convw_d[:])
                nc.sync.dma_start(out=CB[:], in_=convb_d[:])
                nc.sync.dma_start(out=PB[:], in_=primb_d[:])

                # SPIKES[kc] holds conv spikes for all (t, b): [128, 8000]
                SPIKES = [cpool.tile([128, T * BL * 400], bf16, name=f"spk{kc}", tag=f"spk{kc}")
                          for kc in range(2)]
                CONVOUT = [cpool.tile([128, BL * 400], f32, name=f"co{kc}", tag=f"co{kc}")
                           for kc in range(2)]
                MPC = [cpool.tile([128, BL * 400], f32, name=f"mpc{kc}", tag=f"mpc{kc}")
                       for kc in range(2)]
                ASC = [cpool.tile([128, BL * 400], f32, name=f"asc{kc}", tag=f"asc{kc}")
                       for kc in range(2)]

                # --- conv1: out[co, (b,pix)] = sum_k convw[k,co] im2[k,(b,pix)]
                for mc in range(2):
                    for b in range(BL):
                        pc = cps.tile([128, 400], f32, name="cvp", tag="cvp")
                        nc.tensor.matmul(
                            out=pc[:, 0:400],
                            lhsT=CW[:, mc * 128:(mc + 1) * 128],
                            rhs=IM[:, b * 400:(b + 1) * 400],
                            start=True, stop=True)
                        # relu(x + bias): exact via DVE tensor_scalar chain
                        nc.vector.tensor_scalar(
                            out=CONVOUT[mc][:, b * 400:(b + 1) * 400],
                            in0=pc[:, 0:400],
                            scalar1=CB[:, mc:mc + 1], scalar2=0.0,
                            op0=Alu.add, op1=Alu.max)

                # --- conv membrane dynamics for all T steps ---
                for t in range(T):
                    for kc in range(2):
                        spk = A(SPIKES[kc], 0, 128, [(1, BL * 400)],
                                t * BL * 400)
                        if t == 0:
                            nc.vector.tensor_scalar(
                                out=spk, in0=CONVOUT[kc][:],
                                scalar1=1.0, scalar2=None, op0=Alu.is_gt)
                            nc.gpsimd.tensor_tensor(
                                out=MPC[kc][:], in0=CONVOUT[kc][:], in1=spk,
                                op=Alu.subtract)
                        else:
                            nc.vector.scalar_tensor_tensor(
                                out=ASC[kc][:], in0=MPC[kc][:], scalar=0.2,
                                in1=CONVOUT[kc][:],
                                op0=Alu.mult, op1=Alu.add)
                            nc.vector.tensor_scalar(
                                out=spk, in0=ASC[kc][:],
                                scalar1=1.0, scalar2=None, op0=Alu.is_gt)
                            if t < T - 1:
                                nc.gpsimd.tensor_tensor(
                                    out=MPC[kc][:], in0=ASC[kc][:], in1=spk,
                                    op=Alu.subtract)

                if stage == "convmem":
                    nc.gpsimd.dma_start(out=D(outm_d, [(32, 128), (1, 32)]),
                                        in_=A(SPIKES[0], 0, 128, [(1, 32)]))
                    return
                # --- prim conv: batched over all (t, b):
                # out[co, (n=(t,b), oy, ox)] accumulated over (kc, ky, kx)
                PSP = [[pps.tile([128, 360], f32, name=f"pp{mc}{nch}", tag=f"pp{mc}{nch}")
                        for nch in range(2)] for mc in range(2)]
                for pos in range(81):
                    ky, kx = pos // 9, pos % 9
                    wt = wpool.tile([128, 512], bf16, name="w", tag="w")
                    (nc.sync if pos % 2 == 0 else nc.scalar).dma_start(
                        out=wt[:],
                        in_=D(primw_d, [(512, 128), (1, 512)], pos * 128 * 512))
                    for kc in range(2):
                        for mc in range(2):
                            lhsT = wt[:, kc * 256 + mc * 128:
                                      kc * 256 + (mc + 1) * 128]
                            for nch in range(2):
                                rhs = A(SPIKES[kc], 0, 128,
                                        [(400, 10), (40, 6), (2, 6)],
                                        nch * 4000 + ky * 20 + kx)
                                nc.tensor.matmul(
                                    out=PSP[mc][nch][:, 0:360],
                                    lhsT=lhsT, rhs=rhs,
                                    start=(pos == 0 and kc == 0),
                                    stop=(pos == 80 and kc == 1))

                # --- prim evac (+bias), membranes, spikes ---
                PRIM = [cpool.tile([128, T * BL * 36], f32, name=f"pr{mc}", tag=f"pr{mc}")
                        for mc in range(2)]
                PSPK = [cpool.tile([128, T * BL * 36], bf16, name=f"ps{mc}", tag=f"ps{mc}")
                        for mc in range(2)]
                MPP = [cpool.tile([128, BL * 36], f32, name=f"mpp{mc}", tag=f"mpp{mc}")
                       for mc in range(2)]
                APP = [cpool.tile([128, BL * 36], f32, name=f"app{mc}", tag=f"app{mc}")
                       for mc in range(2)]
                for mc in range(2):
                    for nch in range(2):
                        nc.vector.tensor_scalar(
                            out=PRIM[mc][:, nch * 360:(nch + 1) * 360],
                            in0=PSP[mc][nch][:, 0:360],
                            scalar1=PB[:, mc:mc + 1], scalar2=None,
                            op0=Alu.add)
                if stage == "prim":
                    nc.sync.dma_start(out=D(outm_d, [(32, 128), (1, 32)]),
                                      in_=A(PRIM[0], 0, 128, [(1, 32)]))
                    return
                for t in range(T):
                    for mc in range(2):
                        po = A(PRIM[mc], 0, 128, [(1, 144)], t * 144)
                        sp = A(PSPK[mc], 0, 128, [(1, 144)], t * 144)
                        if t == 0:
                            nc.vector.tensor_scalar(
                                out=sp, in0=po, scalar1=1.0, scalar2=None,
                                op0=Alu.is_gt)
                            nc.gpsimd.tensor_tensor(
                                out=MPP[mc][:], in0=po, in1=sp,
                                op=Alu.subtract)
                        else:
                            nc.vector.scalar_tensor_tensor(
                                out=APP[mc][:], in0=MPP[mc][:], scalar=0.2,
                                in1=po, op0=Alu.mult, op1=Alu.add)
                            nc.vector.tensor_scalar(
                                out=sp, in0=APP[mc][:], scalar1=1.0,
                                scalar2=None, op0=Alu.is_gt)
                            if t < T - 1:
                                nc.gpsimd.tensor_tensor(
                                    out=MPP[mc][:], in0=APP[mc][:], in1=sp,
                                    op=Alu.subtract)

                # --- prim spikes -> a2a_in ---
                # flat f = (mc*128+p)*36 + pix ; dest block j = f//1152,
                # rem = f%1152. Split partitions in groups of 32 so j is
                # constant per DMA:  p = ph*32 + pl ->
                # dst = j*23040 + b*5760 + t*1152 + pl*36 + pix, j = mc*4+ph
                env_spk0 = SPIKES[0]
                dmae = [nc.sync, nc.scalar, nc.gpsimd]
                di = 0
                for mc in range(2):
                    for ph in range(4):
                        j = mc * 4 + ph
                        for t in range(T):
                            src = A(PSPK[mc], ph * 32, 32,
                                    [(36, BL), (1, 36)], t * 144)
                            dst = D(a2a_in,
                                    [(36, 32), (5760, BL), (1, 36)],
                                    j * 23040 + t * 1152)
                            dmae[di % 3].dma_start(out=dst, in_=src)
                            di += 1

            if stage == "conv":
                nc.gpsimd.dma_start(out=D(outm_d, [(32, 128), (1, 32)]),
                                    in_=A(env_spk0, 0, 128, [(1, 32)]))
                return
            # =========== AllToAll: batch-shard -> route-shard ===========
            if solo:
                nc.sync.dma_start(out=a2a_out[:], in_=a2a_in[:])
            else:
                nc.gpsimd.collective_compute(
                    "AllToAll", Alu.bypass, replica_groups=rg,
                    ins=[a2a_in[:]], outs=[a2a_out[:]])

            if stage == "a2a":
                nc.gpsimd.dma_start(out=D(outm_d, [(32, 128), (1, 32)]),
                                    in_=D(a2a_out, [(32, 128), (1, 32)]))
                return
            # =========== Phase B prep: transposes + u_hat ===========
            uall = stk.enter_context(tc.tile_pool(name="uall", bufs=1))
            UH_m = uall.tile([128, RL * T * 32], f32, name="uhm", tag="uhm")
            UH_t = uall.tile([128, 36 * T * 32], f32, name="uht", tag="uht")
            with ExitStack() as ustk:
                upool = ustk.enter_context(tc.tile_pool(name="uh", bufs=1))
                w2pool = ustk.enter_context(tc.tile_pool(name="w2c", bufs=3))
                ups = ustk.enter_context(tc.tile_pool(name="upsum", bufs=4,
                                                      space="PSUM"))

                # Load a2a output as M[g] [32 b, (t 5, flat 128)] (contiguous
                # 512B runs), then PE-transpose 45 [32,128] blocks into
                # X16[g] [128 (rr,i), (t,b) 160].
                IDT = upool.tile([32, 32], bf16, name="idt", tag="idt")
                from concourse.masks import make_identity
                make_identity(nc, IDT[:])
                X16 = [persist.tile([128, 160], bf16, name=f"x16_{g}",
                                    tag=f"x16_{g}") for g in range(9)]
                for g in range(9):
                    Mg = upool.tile([32, T * 128], bf16, name="mg", tag="mg",
                                    bufs=2)
                    nc.sync.dma_start(
                        out=Mg[:],
                        in_=D(a2a_out, [(5760, 32), (1152, T), (1, 128)],
                              g * 128))
                    for t in range(T):
                        pst = ups.tile([128, 32], bf16, name="pst", tag="pst", bufs=2)
                        nc.tensor.transpose(
                            out=pst[:], in_=Mg[:, t * 128:(t + 1) * 128],
                            identity=IDT[:])
                        nc.vector.tensor_copy(
                            out=X16[g][:, t * 32:(t + 1) * 32], in_=pst[:])

                if stage == "trans":
                    nc.gpsimd.dma_start(
                        out=D(outm_d, [(32, 128), (1, 32)]),
                        in_=A(X16[0], 0, 128, [(1, 32)]))
                    return
                # u_hat per local route r, kept SBUF-resident for all t:
                #   UH_m [128 co, (r 144, t 5, b 32)]
                #   UH_t [128 (rq,cot), (rl 36, t 5, b 32)]
                # K=32 matmuls with zero-padded weights (partition-aligned);
                # tail matmul writes PSUM at col-group rq*32 so the evac
                # copy is partition-aligned too.
                for g in range(9):
                    w2c = w2pool.tile([128, 16 * CO], bf16, name="w2c",
                                      tag="w2c", bufs=2)
                    (nc.sync if g % 2 == 0 else nc.scalar).dma_start(
                        out=w2c[:],
                        in_=D(w2g_d, [(2560, 128), (1, 2560)],
                              g * 128 * 2560))
                    for r2 in range(8):
                        psA = ups.tile([128, 320], f32, name="upa", tag="upa",
                                       bufs=3)
                        psB = ups.tile([128, 320], f32, name="upb", tag="upb",
                                       bufs=3)
                        for j in range(2):
                            rr = r2 * 2 + j
                            r = g * 16 + rr
                            rq = r // 36
                            q = (rr // 4) * 32
                            rhs = A(X16[g], q, 32, [(1, 160)])
                            nc.tensor.matmul(
                                out=psA[:, j * 160:(j + 1) * 160],
                                lhsT=A(w2c, q, 32, [(1, 128)], rr * CO),
                                rhs=rhs, start=True, stop=True,
                                tile_position=(q, 0))
                            nc.tensor.matmul(
                                out=A(psB, rq * 32, 32, [(1, 160)], j * 160),
                                lhsT=A(w2c, q, 32, [(1, 32)], rr * CO + 128),
                                rhs=rhs, start=True, stop=True,
                                tile_position=(q, rq * 32))
                        r0 = g * 16 + r2 * 2
                        rq0, rl0 = r0 // 36, r0 % 36
                        if r2 % 2 == 0:
                            nc.vector.tensor_copy(
                                out=A(UH_m, 0, 128, [(1, 320)], r0 * 160),
                                in_=psA[:, 0:320])
                            nc.scalar.copy(
                                out=A(UH_t, rq0 * 32, 32, [(1, 320)],
                                      rl0 * 160),
                                in_=A(psB, rq0 * 32, 32, [(1, 320)]))
                        else:
                            nc.scalar.copy(
                                out=A(UH_m, 0, 128, [(1, 320)], r0 * 160),
                                in_=psA[:, 0:320])
                            nc.vector.tensor_copy(
                                out=A(UH_t, rq0 * 32, 32, [(1, 320)],
                                      rl0 * 160),
                                in_=A(psB, rq0 * 32, 32, [(1, 320)]))

            if stage == "uhat":
                nc.gpsimd.dma_start(out=D(outm_d, [(32, 128), (1, 32)]),
                                    in_=A(X16[0], 0, 128, [(1, 32)]))
                return
            # =========== Phase B: digit-caps loop (route-sharded) ========
            dpool = stk.enter_context(tc.tile_pool(name="dig", bufs=1))
            dups = stk.enter_context(tc.tile_pool(name="dups", bufs=2))
            dps = stk.enter_context(tc.tile_pool(name="dpsum", bufs=2,
                                                 space="PSUM"))

            NM = RL * 32            # 4608
            NTT = 36 * 32           # 1152
            THRM = dpool.tile([128, T], f32, name="thrm", tag="thrm")
            THRT = dpool.tile([128, T], f32, name="thrt", tag="thrt")
            SELA = dpool.tile([128, 160], f32, name="sela", tag="sela")
            SELT = dpool.tile([32, 160], f32, name="selt", tag="selt")
            nc.sync.dma_start(out=THRM[:], in_=thrm_d[:])
            nc.sync.dma_start(out=THRT[:], in_=thrt_d[:])
            nc.sync.dma_start(out=SELA[:], in_=sela_d[:])
            nc.sync.dma_start(out=SELT[:], in_=selt_d[:])

            MD_m = dpool.tile([128, NM], f32, name="mdm", tag="mdm")
            MD_t = dpool.tile([128, NTT], f32, name="mdt", tag="mdt")
            DS_m = dpool.tile([128, NM], f32, name="dsm", tag="dsm")
            DS_t = dpool.tile([128, NTT], f32, name="dst", tag="dst")
            TR_m = dpool.tile([128, NM], f32, name="trm", tag="trm")
            TR_t = dpool.tile([128, NTT], f32, name="trt", tag="trt")
            BIJ_m = dpool.tile([128, RL], f32, name="bijm", tag="bijm")
            BIJ_t = dpool.tile([128, 36], f32, name="bijt", tag="bijt")
            ZB_m = dpool.tile([128, RL], f32, name="zbm", tag="zbm")
            ZB_t = dpool.tile([128, 36], f32, name="zbt", tag="zbt")
            SJQ = dpool.tile([128, 32], f32, name="sjq", tag="sjq")
            SJF_m = dpool.tile([128, 32], f32, name="sjfm", tag="sjfm")
            SJF_t = dpool.tile([32, 32], f32, name="sjft", tag="sjft")
            A2_m = dpool.tile([128, 32], f32, name="a2m", tag="a2m")
            A2_t = dpool.tile([32, 32], f32, name="a2t", tag="a2t")
            M2_m = dpool.tile([128, 32], f32, name="m2m", tag="m2m")
            M2_t = dpool.tile([32, 32], f32, name="m2t", tag="m2t")
            D2_m = dpool.tile([128, 32], f32, name="d2m", tag="d2m")
            D2_t = dpool.tile([32, 32], f32, name="d2t", tag="d2t")
            D2F = dpool.tile([128, 32], f32, name="d2f", tag="d2f")
            DBG = dpool.tile([128, 512], f32, name="dbg", tag="dbg")
            OUT_m = dpool.tile([128, 32], f32, name="outm", tag="outm")
            OUT_t = dpool.tile([32, 32], f32, name="outt", tag="outt")
            DSU_m = dpool.tile([128, 1], f32, name="dsum", tag="dsum")
            DSU_t = dpool.tile([32, 1], f32, name="dsut", tag="dsut")
            SCR_m = dpool.tile([128, 32], f32, name="scrm", tag="scrm")
            SCR_t = dpool.tile([32, 32], f32, name="scrt", tag="scrt")
            DPDF = dpool.tile([128, 36], f32, name="dpdf", tag="dpdf")
            SCOLF = dpool.tile([128, 1], f32, name="scolf", tag="scolf")

            bij0 = float(np.float32(1.0) / np.float32(R))
            nc.vector.memset(BIJ_m[:], bij0)
            nc.vector.memset(BIJ_t[:], bij0)

            for t in range(T):
                # ---- u_hat slices for this step (strided SBUF views) ----
                Um = A(UH_m, 0, 128, [(5 * 32, RL), (1, 32)], t * 32)
                Ut = A(UH_t, 0, 128, [(5 * 32, 36), (1, 32)], t * 32)
                DSm = A(DS_m, 0, 128, [(32, RL), (1, 32)])
                DSt = A(DS_t, 0, 128, [(32, 36), (1, 32)])
                MDm = A(MD_m, 0, 128, [(32, RL), (1, 32)])
                MDt = A(MD_t, 0, 128, [(32, 36), (1, 32)])

                # ---- membrane a-pass (in place on U), spikes, reset ----
                if t > 0:
                    nc.vector.scalar_tensor_tensor(
                        out=Um, in0=MDm, scalar=0.2, in1=Um,
                        op0=Alu.mult, op1=Alu.add)
                    nc.vector.scalar_tensor_tensor(
                        out=Ut, in0=MDt, scalar=0.2, in1=Ut,
                        op0=Alu.mult, op1=Alu.add)
                nc.vector.tensor_scalar(
                    out=DSm, in0=Um, scalar1=THRM[:, t:t + 1],
                    scalar2=None, op0=Alu.is_gt)
                nc.vector.tensor_scalar(
                    out=DSt, in0=Ut, scalar1=THRT[:, t:t + 1],
                    scalar2=None, op0=Alu.is_gt)
                if t < T - 1:
                    nc.gpsimd.tensor_tensor(
                        out=MDm, in0=Um, in1=DSm, op=Alu.subtract)
                    nc.gpsimd.tensor_tensor(
                        out=MDt, in0=Ut, in1=DSt, op=Alu.subtract)

                # ---- trace update (needed for steps 0..T-2) ----
                if t == 0:
                    nc.scalar.copy(out=TR_m[:], in_=DS_m[:])
                    nc.scalar.copy(out=TR_t[:], in_=DS_t[:])
                elif t < T - 1:
                    nc.vector.scalar_tensor_tensor(
                        out=TR_m[:], in0=TR_m[:], scalar=float(DECAY_TR),
                        in1=DS_m[:], op0=Alu.mult, op1=Alu.max)
                    nc.vector.scalar_tensor_tensor(
                        out=TR_t[:], in0=TR_t[:], scalar=float(DECAY_TR),
                        in1=DS_t[:], op0=Alu.mult, op1=Alu.max)

                # ---- y = ds * bij ; s_j partials (reduce over r) ----
                nc.vector.tensor_tensor(
                    out=DSm, in0=DSm,
                    in1=A(BIJ_m, 0, 128, [(1, RL), (0, 32)]),
                    op=Alu.mult)
                nc.vector.tensor_tensor(
                    out=DSt, in0=DSt,
                    in1=A(BIJ_t, 0, 128, [(1, 36), (0, 32)]),
                    op=Alu.mult)
                nc.vector.tensor_reduce(
                    out=SJF_m[:], in_=A(DS_m, 0, 128, [(1, 32), (32, RL)]),
                    axis=mybir.AxisListType.X, op=Alu.add)
                nc.vector.tensor_reduce(
                    out=SJQ[:], in_=A(DS_t, 0, 128, [(1, 32), (32, 36)]),
                    axis=mybir.AxisListType.X, op=Alu.add)

                # ---- exchange s_j partials (AllGather + local sum) ----
                nc.sync.dma_start(out=D(sj_in[t], [(32, 128), (1, 32)]),
                                  in_=SJF_m[:])
                nc.sync.dma_start(out=D(sj_in[t], [(32, 128), (1, 32)], 4096),
                                  in_=SJQ[:])
                if solo:
                    nc.sync.dma_start(
                        out=D(sj_out[t], [(1, SJP)]), in_=sj_in[t][:])
                else:
                    nc.gpsimd.collective_compute(
                        "AllGather", Alu.bypass, replica_groups=rg,
                        ins=[sj_in[t][:]], outs=[sj_out[t][:]])
                SJG_m = dups.tile([128, 8 * 32], f32, name="sjgm", tag="sjgm")
                SJG_t = dups.tile([32, 4 * 8 * 32], f32, name="sjgt", tag="sjgt")
                nc.sync.dma_start(
                    out=A(SJG_m, 0, 128, [(32, 8), (1, 32)]),
                    in_=D(sj_out[t], [(32, 128), (SJP, 8), (1, 32)]))
                for rq in range(4):
                    nc.sync.dma_start(
                        out=A(SJG_t, 0, 32, [(128, 8), (1, 32)], rq * 32),
                        in_=D(sj_out[t], [(32, 32), (SJP, 8), (1, 32)],
                              4096 + rq * 1024))
                nc.vector.tensor_reduce(
                    out=SJF_m[:], in_=A(SJG_m, 0, 128, [(1, 32), (32, 8)]),
                    axis=mybir.AxisListType.X, op=Alu.add)
                nc.vector.tensor_reduce(
                    out=SJF_t[:],
                    in_=A(SJG_t, 0, 32, [(1, 32), (32, 8 * 4)]),
                    axis=mybir.AxisListType.X, op=Alu.add)

                # ---- dig2 membranes, out accumulation ----
                if t == 0:
                    a2m, a2t = SJF_m, SJF_t
                    nc.vector.tensor_copy(out=OUT_m[:], in_=SJF_m[:])
                    nc.vector.tensor_copy(out=OUT_t[:], in_=SJF_t[:])
                else:
                    nc.vector.scalar_tensor_tensor(
                        out=A2_m[:], in0=M2_m[:], scalar=0.2, in1=SJF_m[:],
                        op0=Alu.mult, op1=Alu.add)
                    nc.vector.scalar_tensor_tensor(
                        out=A2_t[:], in0=M2_t[:], scalar=0.2, in1=SJF_t[:],
                        op0=Alu.mult, op1=Alu.add)
                    a2m, a2t = A2_m, A2_t
                    nc.vector.tensor_tensor(out=OUT_m[:], in0=OUT_m[:],
                                            in1=SJF_m[:], op=Alu.add)
                    nc.vector.tensor_tensor(out=OUT_t[:], in0=OUT_t[:],
                                            in1=SJF_t[:], op=Alu.add)
                if t < T - 1:
                    nc.vector.tensor_scalar(
                        out=D2_m[:], in0=a2m[:], scalar1=0.5, scalar2=None,
                        op0=Alu.is_gt)
                    nc.vector.tensor_scalar(
                        out=D2_t[:], in0=a2t[:], scalar1=0.5, scalar2=None,
                        op0=Alu.is_gt)
                    nc.vector.scalar_tensor_tensor(
                        out=M2_m[:], in0=D2_m[:], scalar=-0.5, in1=a2m[:],
                        op0=Alu.mult, op1=Alu.add)
                    nc.vector.scalar_tensor_tensor(
                        out=M2_t[:], in0=D2_t[:], scalar=-0.5, in1=a2t[:],
                        op0=Alu.mult, op1=Alu.add)

                    # d2s tail-folded copy (partition realign via DMA)
                    for rq in range(4):
                        nc.sync.dma_start(
                            out=A(D2F, rq * 32, 32, [(1, 32)]),
                            in_=D2_t[:])

                    # ---- z = trace * d2s (onto DS); zb = sum_b z ----
                    nc.vector.tensor_tensor(
                        out=DSm,
                        in0=A(TR_m, 0, 128, [(32, RL), (1, 32)]),
                        in1=A(D2_m, 0, 128, [(0, RL), (1, 32)]),
                        op=Alu.mult)
                    nc.vector.tensor_tensor(
                        out=DSt,
                        in0=A(TR_t, 0, 128, [(32, 36), (1, 32)]),
                        in1=A(D2F, 0, 128, [(0, 36), (1, 32)]),
                        op=Alu.mult)
                    nc.vector.tensor_reduce(
                        out=ZB_m[:], in_=A(DS_m, 0, 128, [(32, RL), (1, 32)]),
                        axis=mybir.AxisListType.X, op=Alu.add)
                    nc.vector.tensor_reduce(
                        out=ZB_t[:], in_=A(DS_t, 0, 128, [(32, 36), (1, 32)]),
                        axis=mybir.AxisListType.X, op=Alu.add)

                    # scaled d2s sums for the -0.1*alpha*S[c] term
                    nc.vector.tensor_scalar(
                        out=SCR_m[:], in0=D2_m[:],
                        scalar1=float(np.float32(0.1) * ALPHA), scalar2=None,
                        op0=Alu.mult, op1=Alu.add, accum_out=DSU_m[:])
                    nc.vector.tensor_scalar(
                        out=SCR_t[:], in0=D2_t[:],
                        scalar1=float(np.float32(0.1) * ALPHA), scalar2=None,
                        op0=Alu.mult, op1=Alu.add, accum_out=DSU_t[:])

                    if t == 0:
                        nc.vector.tensor_copy(out=DBG[:, 0:144],
                                              in_=ZB_m[:])
                        nc.vector.tensor_copy(out=DBG[:, 144:180],
                                              in_=ZB_t[:])
                    # ---- delta matmuls: PD[co', r] = sum_co sel * zb ----
                    PD_m = dps.tile([128, 145], f32, name="pdm", tag="pdm")
                    PD_t = dps.tile([32, 145], f32, name="pdt", tag="pdt")
                    nc.tensor.matmul(out=PD_m[:, 0:144],
                                     lhsT=SELA[:, 0:128], rhs=ZB_m[:],
                                     start=True, stop=False)
                    if t == 0:
                        nc.vector.tensor_copy(out=DBG[:, 145:289], in_=PD_m[:, 0:144])
                    nc.tensor.matmul(out=PD_m[:, 144:145],
                                     lhsT=SELA[:, 0:128], rhs=DSU_m[:],
                                     start=True, stop=False)
                    nc.tensor.matmul(out=PD_t[:, 0:144],
                                     lhsT=SELA[:, 128:160], rhs=ZB_m[:],
                                     start=True, stop=False)
                    nc.tensor.matmul(out=PD_t[:, 144:145],
                                     lhsT=SELA[:, 128:160], rhs=DSU_m[:],
                                     start=True, stop=False)
                    if t == 0:
                        nc.vector.tensor_copy(out=DBG[:, 180:212],
                                              in_=D2_m[:])
                        nc.vector.tensor_copy(out=DBG[0:32, 212:244],
                                              in_=D2_t[:])
                    # partition-compress ZB_t [128 (rq,cot), 36] ->
                    # ZBT4 [32 cot, (rq, 36)] so the matmul K sits at base 0
                    ZBT4 = dups.tile([32, 144], f32, name="zbt4", tag="zbt4")
                    for rq in range(4):
                        nc.sync.dma_start(
                            out=A(ZBT4, 0, 32, [(1, 36)], rq * 36),
                            in_=A(ZB_t, rq * 32, 32, [(1, 36)]))
                    nc.tensor.matmul(out=PD_m[:, 0:144],
                                     lhsT=SELT[:, 0:128], rhs=ZBT4[:],
                                     start=False, stop=False)
                    nc.tensor.matmul(out=PD_t[:, 0:144],
                                     lhsT=SELT[:, 128:160], rhs=ZBT4[:],
                                     start=False, stop=False)
                    nc.tensor.matmul(out=PD_m[:, 144:145],
                                     lhsT=SELT[:, 0:128], rhs=DSU_t[:],
                                     start=False, stop=True)
                    nc.tensor.matmul(out=PD_t[:, 144:145],
                                     lhsT=SELT[:, 128:160], rhs=DSU_t[:],
                                     start=False, stop=True)

                    if t == 0:
                        nc.vector.tensor_copy(out=DBG[:, 0:144],
                                              in_=PD_m[:, 0:144])
                        nc.vector.tensor_copy(out=DBG[:, 144:145],
                                              in_=PD_m[:, 144:145])
                    # ---- bij updates ----
                    nc.vector.scalar_tensor_tensor(
                        out=BIJ_m[:], in0=PD_m[:, 0:144], scalar=float(ALPHA),
                        in1=BIJ_m[:], op0=Alu.mult, op1=Alu.add)
                    nc.vector.tensor_scalar(
                        out=BIJ_m[:], in0=BIJ_m[:],
                        scalar1=PD_m[:, 144:145], scalar2=None,
                        op0=Alu.subtract)
                    # tail: fold [32 co', 144 r] -> [128 (rq,cot), 36]
                    PDTS = dups.tile([32, 145], f32, name="pdts", tag="pdts")
                    nc.vector.tensor_copy(out=PDTS[:], in_=PD_t[:])
                    for rq in range(4):
                        nc.sync.dma_start(
                            out=A(DPDF, rq * 32, 32, [(1, 36)]),
                            in_=A(PDTS, 0, 32, [(1, 36)], rq * 36))
                        nc.sync.dma_start(
                            out=A(SCOLF, rq * 32, 32, [(1, 1)]),
                            in_=PDTS[:, 144:145])
                    nc.vector.scalar_tensor_tensor(
                        out=BIJ_t[:], in0=DPDF[:], scalar=float(ALPHA),
                        in1=BIJ_t[:], op0=Alu.mult, op1=Alu.add)
                    nc.vector.tensor_scalar(
                        out=BIJ_t[:], in0=BIJ_t[:], scalar1=SCOLF[:],
                        scalar2=None, op0=Alu.subtract)

            nc.sync.dma_start(out=dbg_d[:], in_=DBG[:])
            # ---- write outputs ----
            nc.sync.dma_start(out=D(outm_d, [(32, 128), (1, 32)]),
                              in_=OUT_m[:])
            nc.sync.dma_start(out=D(outm_d, [(32, 32), (1, 32)], 128 * 32),
                              in_=OUT_t[:])


def _host_prepare(data, conv_w, conv_b, prim_w, prim_b, W, bias):
    """Build per-core input maps."""
    from numpy.lib.stride_tricks import sliding_window_view
    f32 = np.float32
    data = np.asarray(data, f32)
    conv_w = np.asarray(conv_w, f32)
    conv_b = np.asarray(conv_b, f32)
    prim_w = np.asarray(prim_w, f32)
    prim_b = np.asarray(prim_b, f32)
    W = np.asarray(W, f32)
    bias = np.asarray(bias, f32)

    # im2col: win[b, ky, kx, oy, ox]
    win = sliding_window_view(data[:, 0, :, :], (20, 20), axis=(1, 2))
    im2_all = np.ascontiguousarray(win).reshape(B, 81, 400)

    # everything feeding the spiking membranes runs in a 2x-scaled domain
    # (exact in fp32) so the reset is the plain subtract M = A - ds.
    convw = np.ascontiguousarray(conv_w.reshape(256, 81).T) * f32(2.0)
    convb2 = np.ascontiguousarray(conv_b.reshape(2, 128).T) * f32(2.0)

    import ml_dtypes
    bf16 = ml_dtypes.bfloat16
    pw = prim_w.reshape(2, 128, 2, 128, 9, 9)
    primw = (np.ascontiguousarray(
        pw.transpose(4, 5, 3, 2, 0, 1).reshape(81, 128, 512))
        * f32(2.0)).astype(bf16)
    primb2 = np.ascontiguousarray(prim_b.reshape(2, 128).T) * f32(2.0)

    # W2[i, r, co] with co = o*10 + c, zero-padded to K=32 route-quads:
    # w2g[g, rr*8+i, rr*160+co] = 2*W2[i, g*16+rr, co]
    Wt = np.ascontiguousarray(
        W.transpose(3, 0, 2, 1)).reshape(8, R, CO) * f32(2.0)

    # per-step thresholds: thr_t[co] = 0.5 - b_t[o], b_t = 0.2*b_{t-1}+bias_o
    bias_o = bias[:, 0]
    thr = np.zeros((CO, T), f32)
    bt = bias_o.copy()
    for t in range(T):
        for co in range(CO):
            thr[co, t] = np.float32(1.0) - f32(2.0) * bt[co // 10]
        bt = (f32(0.2) * bt + bias_o).astype(f32)
    thrm = np.ascontiguousarray(thr[:128])
    # tail thresholds, folded to 128 partitions (cot = p % 32)
    thrt = np.zeros((128, T), f32)
    for p in range(128):
        thrt[p] = thr[128 + p % 32]

    cos = np.arange(CO)
    sela = (np.equal.outer(cos[:128] % 10, cos % 10)).astype(f32)
    selt = (np.equal.outer(cos[128:] % 10, cos % 10)).astype(f32)
    sela = np.ascontiguousarray(sela)
    selt = np.ascontiguousarray(selt)

    in_maps = []
    for k in range(N_CORES):
        im2 = np.ascontiguousarray(
            im2_all[BL * k:BL * (k + 1)].transpose(1, 0, 2).reshape(81, 1600))
        w2core = Wt[:, RL * k:RL * (k + 1), :]          # [8, 144, 160]
        w2g = np.zeros((9, 128, 16 * CO), bf16)
        for rr in range(16):
            # [8, 9, 160] block for this rr across all 9 groups
            blk = w2core[:, rr::16, :]
            w2g[:, rr * 8:(rr + 1) * 8, rr * CO:(rr + 1) * CO] = \
                blk.transpose(1, 0, 2)
        in_maps.append({
            "im2": im2, "convw": convw, "convb": convb2,
            "primw": primw, "primb": primb2, "w2g": w2g,
            "thrm": thrm, "thrt": thrt, "sela": sela, "selt": selt,
        })
    return in_maps


def _postprocess(outm):
    """outm [160, 32] (co = o*10+c) -> classes [32, 10]."""
    out3 = outm.reshape(16, 10, 32).astype(np.float32) / np.float32(T)
    sq = (out3 * out3).sum(axis=0)
    return np.sqrt(sq).T.astype(np.float32)


def kernel(data, conv_w, conv_b, prim_w, prim_b, W, bias, time_window):
    from concourse.bass_utils import run_bass_kernel_spmd
    assert int(time_window) == T
    if "nc" not in _CACHE:
        _CACHE["nc"] = _build_program()
    nc = _CACHE["nc"]
    in_maps = _host_prepare(data, conv_w, conv_b, prim_w, prim_b, W, bias)
    res = run_bass_kernel_spmd(nc, in_maps, core_ids=list(range(N_CORES)))
    return _postprocess(res.results[0]["outm"])

